# revision 8
# baseline (speedup 1.0000x reference)
"""Trainium2 Bass kernel for the CN coupling-block problem (nn_CN_69312182223156).

Math (per subnet s on half-features x_s with conditioner c):
    h   = relu(c @ W1 + b1)                       # [B, 50]
    p   = h @ W2 + b2                             # [B, 9696]
    m1, b1p, m2 = p[:, :3200], p[:, 3200:6400], p[:, 6400:9600]   (viewed [B,32,100])
    bias2, eps, alpha = p[:, 9600:9632], p[:, 9632:9664]/10, p[:, 9664:]/10
    z   = x*m1 + b1p
    num = sum_l elu(z)*m2 ;  den = sum_l relu(-m1*m2) + 1
    y   = exp(alpha) * (x + 0.8*sigmoid(eps)*num/den) + bias2

Subnet 1: x=x1, c=x2.  Subnet 2: x=x2, c=y1.  Output concat([y1, y2]).

Strategy: pure data-parallel over 8 cores (2048 rows each), weights replicated.
Layout: batch on SBUF partitions (tiles of 128 rows); the [B, 9696] parameter
tensor is produced on PE in 800-column chunks (8 dims x 100) and consumed
immediately.  Work is spread over three engines (the old version used two):
  ACT: PSUM->SBUF f16 casts (m1+m2 merged into one strided op, then b1) + exp
  DVE: per-dim x-broadcast tensor_scalar (4x), z1 add, min, max, w*m2, the
       final reduce, and the den reduction as 8 per-dim
       tensor_scalar(min 0, add 0) with fused accum_out (the hardware
       accumulator applies op1, so op1 must be the add; the -1 is applied in
       the tail).
  GPSIMD: u = m1*m2 and the two num pair-fold adds (only plain TT-class SBUF
       ops lower on Pool).
The per-chunk work is emitted as a 5-deep software pipeline (stages A-E over
chunk index) so that every instruction is data-ready when it reaches the head
of its engine's in-order queue: A(i)=matmuls+casts+u, B(i-1)=z-chain+exp,
C(i-2)=w,w*m2, E(i-3)=folds, D(i-4)=reduce+den.  Tails (bias2|eps|alpha|S2,
num/den combine) are batched over groups of 4 row-tiles.
Biases fold into augmented weights; S2 = sum_l mat2 rides as 32 extra matmul
columns so num needs no -1 term:  elu(z)+1 = max(z+1, exp(min(z,0))).
"""

import numpy as np

B = 16384
DIM = 32
LS = 100
NCORES = 8
BC = B // NCORES          # rows per core
NT = BC // 128            # 128-row tiles per core
GT = 4                    # tiles per tail group
DL = DIM * LS             # 3200
PW = 3 * DL + 3 * DIM     # 9696 params per row
CHUNK = 800               # params per elementwise chunk (8 dims x 100)
HALF = 400                # params per PSUM-bank matmul
NCHUNK = DL // CHUNK      # 4
DPC = CHUNK // LS         # 8 dims per chunk

_cache = {}


def _build_program():
    import concourse.bass as bass
    import concourse.tile as tile
    import concourse.mybir as mybir
    from concourse import bacc, masks

    f32 = mybir.dt.float32
    f16 = mybir.dt.float16
    Alu = mybir.AluOpType
    Act = mybir.ActivationFunctionType
    X = mybir.AxisListType.X

    nc = bacc.Bacc("TRN2", target_bir_lowering=False)

    x_d = nc.dram_tensor("x", [BC, 2 * DIM], f32, kind="ExternalInput")
    w1a = [nc.dram_tensor(f"w1a{s}", [DIM + 1, 51], f16, kind="ExternalInput")
           for s in (1, 2)]
    w2a = [nc.dram_tensor(f"w2a{s}", [51, PW + DIM], f16, kind="ExternalInput")
           for s in (1, 2)]
    y_d = nc.dram_tensor("y", [BC, 2 * DIM], f32, kind="ExternalOutput")

    with tile.TileContext(nc) as tc:
        with (
            tc.tile_pool(name="const", bufs=1) as const,
            tc.tile_pool(name="per", bufs=1) as per,
            tc.tile_pool(name="mid", bufs=4) as mid,
            tc.tile_pool(name="hts", bufs=8) as hts,
            tc.tile_pool(name="ew", bufs=6) as ew,
            tc.tile_pool(name="tailp", bufs=2) as tailp,
            tc.tile_pool(name="pmm1", bufs=1, space="PSUM") as pmm1,
            tc.tile_pool(name="pmm2", bufs=1, space="PSUM") as pmm2,
            tc.tile_pool(name="psm", bufs=2, space="PSUM") as psm,
        ):
            # ---- constants ----
            w1s = []
            w2s = []
            for s in range(2):
                t1 = const.tile([DIM + 1, 51], f16, tag=f"w1_{s}", name="t1")
                nc.sync.dma_start(t1, w1a[s][:])
                w1s.append(t1)
                t2 = const.tile([51, PW + DIM], f16, tag=f"w2_{s}", name="t2")
                nc.sync.dma_start(t2, w2a[s][:])
                w2s.append(t2)
            identf = const.tile([128, 128], f32, tag="identf", name="identf")
            masks.make_identity(nc, identf[:])
            negone = const.tile([128, 1], f32, tag="negone", name="negone")
            nc.vector.memset(negone, -1.0)

            xfs, youts, nd4s, hTd = {}, {}, {}, {}

            def prelude(s, it):
                """Per-tile setup: x load (phase 1), conditioner transpose,
                h matmul, hT relu-cast."""
                if s == 0:
                    r0 = it * 128
                    xf = per.tile([128, 2 * DIM + 1], f32, tag=f"xf{it}",
                                  name="xf")
                    nc.sync.dma_start(xf[:, 0:2 * DIM], x_d[r0:r0 + 128, :])
                    nc.vector.memset(xf[:, 2 * DIM:], 1.0)
                    xfs[it] = xf
                    y_out = per.tile([128, 2 * DIM], f32, tag=f"y_out{it}",
                                     name="y_out")
                    youts[it] = y_out
                    # conditioner for subnet 1: [x2 | 1]^T  -> [33, 128]
                    ct_ps = psm.tile([DIM + 1, 128], f32, tag="tp", name="ct_ps")
                    nc.tensor.transpose(ct_ps, xf[:, DIM:2 * DIM + 1], identf)
                    condT = mid.tile([DIM + 1, 128], f16, tag="condT",
                                     name="condT")
                    nc.scalar.copy(condT, ct_ps)
                else:
                    y_out = youts[it]
                    # conditioner for subnet 2: [y1 | 1]^T
                    c2_ps = psm.tile([DIM, 128], f32, tag="tp", name="c2_ps")
                    nc.tensor.transpose(c2_ps, y_out[:, 0:DIM], identf)
                    condT = mid.tile([DIM + 1, 128], f16, tag="condT",
                                     name="condT2")
                    nc.scalar.copy(condT[0:DIM, :], c2_ps)
                    nc.vector.memset(condT[DIM:DIM + 1, :], 1.0)
                # h^T = relu(W1^T c^T + b1): [51, 128]; col 50 of W1aug is
                # e_32 so row 50 comes out as relu(1) = 1 (the aug ones row).
                h_ps = psm.tile([51, 128], f32, tag="tp", name="h_ps")
                nc.tensor.matmul(h_ps, w1s[s], condT, start=True, stop=True)
                hT = hts.tile([51, 128], f16, tag="hT", name="hT")
                nc.scalar.activation(hT, h_ps, Act.Relu)
                hTd[(s, it)] = hT
                g, itg = it // GT, it % GT
                if itg == 0:
                    nd4s[(s, g)] = per.tile([128, GT, 2, DIM], f32,
                                            tag=f"nd{s}_{g}", name="nd4")

            def stage_a(s, it, c, st):
                """PE matmuls into PSUM, ACT casts (m1+m2 first so the Pool's
                u can start early, then b1), Pool u = m1*m2."""
                hT = hTd[(s, it)]
                co = c * CHUNK
                p1 = pmm1.tile([128, 4, 512], f32, tag="p1", name="p1")
                p2 = pmm2.tile([128, 2, 512], f32, tag="p2", name="p2")
                for hh in range(2):
                    o = co + hh * HALF
                    nc.tensor.matmul(p1[:, hh, 0:HALF], hT,
                                     w2s[s][:, o:o + HALF],
                                     start=True, stop=True)
                    nc.tensor.matmul(p1[:, 2 + hh, 0:HALF], hT,
                                     w2s[s][:, 2 * DL + o:2 * DL + o + HALF],
                                     start=True, stop=True)
                    nc.tensor.matmul(p2[:, hh, 0:HALF], hT,
                                     w2s[s][:, DL + o:DL + o + HALF],
                                     start=True, stop=True)
                mm12 = ew.tile([128, 4, HALF], f16, tag="mm12", name="mm12")
                nc.scalar.copy(mm12, p1[:, :, 0:HALF])
                b1t = ew.tile([128, 2, HALF], f16, tag="b1t", name="b1t")
                nc.scalar.copy(b1t, p2[:, :, 0:HALF])
                st["m1f"] = mm12[:, 0:2, :].rearrange("p h q -> p (h q)")
                st["m2f"] = mm12[:, 2:4, :].rearrange("p h q -> p (h q)")
                st["b1f"] = b1t.rearrange("p h q -> p (h q)")
                u = ew.tile([128, CHUNK], f16, tag="u", name="u")
                nc.gpsimd.tensor_mul(u, st["m1f"], st["m2f"])
                st["u"] = u

            def stage_b(s, it, c, st):
                """z = x*m1 (+ b1+1), capped arg, exp."""
                xf = xfs[it]
                xc32 = xf[:, s * DIM:(s + 1) * DIM]
                zmul = ew.tile([128, CHUNK], f16, tag="zmul", name="zmul")
                zm3 = zmul.rearrange("p (d l) -> p d l", l=LS)
                m1s3 = st["m1f"].rearrange("p (d l) -> p d l", l=LS)
                for j in range(DPC):
                    nc.vector.tensor_scalar_mul(
                        zm3[:, j, :], m1s3[:, j, :],
                        xc32[:, c * DPC + j:c * DPC + j + 1])
                # b1f carries b1+1, so z1 = z+1 and, using e^x >= 1+x:
                #   elu(z)+1 = max(z+1, exp(min(z,0)))
                z1 = ew.tile([128, CHUNK], f16, tag="z1", name="z1")
                nc.vector.tensor_add(z1, zmul, st["b1f"])
                zn = ew.tile([128, CHUNK], f16, tag="zn", name="zn")
                nc.vector.tensor_scalar_min(zn, z1, 1.0)
                e = ew.tile([128, CHUNK], f16, tag="e", name="e")
                nc.scalar.activation(e, zn, Act.Exp, bias=negone)
                st["z1"], st["e"] = z1, e

            def stage_c(s, it, c, st):
                """w = elu(z)+1 = max(z1, e);  t = w*m2."""
                w = ew.tile([128, CHUNK], f16, tag="w", name="w")
                nc.vector.tensor_tensor(w, st["z1"], st["e"], Alu.max)
                tr0 = ew.tile([128, DPC, LS], f16, tag="tr0", name="tr0")
                nc.vector.tensor_mul(
                    tr0, w.rearrange("p (d l) -> p d l", l=LS),
                    st["m2f"].rearrange("p (d l) -> p d l", l=LS))
                st["tr0"] = tr0

            def stage_e(s, it, c, st):
                """Two pair-fold adds for num, on the Pool engine."""
                tr4 = st["tr0"].rearrange("p d (f l) -> p d f l", f=2)
                th = ew.tile([128, DPC, LS // 2], f16, tag="th", name="th")
                nc.gpsimd.tensor_add(th, tr4[:, :, 0, :], tr4[:, :, 1, :])
                th4 = th.rearrange("p d (f l) -> p d f l", f=2)
                th2 = ew.tile([128, DPC, LS // 4], f16, tag="th2", name="th2")
                nc.gpsimd.tensor_add(th2, th4[:, :, 0, :], th4[:, :, 1, :])
                st["th2"] = th2

            def stage_d(s, it, c, st):
                """Final num reduce; den via per-dim min/add with fused
                accumulate (accum applies op1=add; sign fixed in the tail:
                sum_l relu(-u) = -sum_l min(u, 0))."""
                g, itg = it // GT, it % GT
                nd4 = nd4s[(s, g)]
                numo = nd4[:, itg, 0, c * DPC:(c + 1) * DPC]
                nc.vector.tensor_reduce(numo, st["th2"], X, Alu.add)
                scr = ew.tile([128, DPC, LS], f16, tag="scr", name="scr")
                u3 = st["u"].rearrange("p (d l) -> p d l", l=LS)
                for j in range(DPC):
                    dd = c * DPC + j
                    nc.vector.tensor_scalar(
                        scr[:, j, :], u3[:, j, :], 0.0, 0.0,
                        Alu.min, Alu.add,
                        accum_out=nd4[:, itg, 1, dd:dd + 1])

            def tail_group(s, g):
                """Batched tail for GT tiles: tail matmuls into one PSUM bank,
                elementwise at FD=128 on [128, GT, DIM] views."""
                nd4 = nd4s[(s, g)]
                tp4 = psm.tile([128, 4, 128], f32, tag="tp", name="tp4")
                for t in range(GT):
                    nc.tensor.matmul(tp4[:, t, :], hTd[(s, g * GT + t)],
                                     w2s[s][:, 3 * DL:3 * DL + 4 * DIM],
                                     start=True, stop=True)
                b2p = tp4[:, :, 0:DIM]
                epp = tp4[:, :, DIM:2 * DIM]
                alp = tp4[:, :, 2 * DIM:3 * DIM]
                s2p = tp4[:, :, 3 * DIM:4 * DIM]
                num4 = nd4[:, :, 0, :]
                den4 = nd4[:, :, 1, :]

                den = tailp.tile([128, GT, DIM], f32, tag="den", name="den")
                nc.vector.tensor_scalar(den, den4, -1.0, 1.0, Alu.mult, Alu.add)
                rec = tailp.tile([128, GT, DIM], f32, tag="rec", name="rec")
                nc.vector.reciprocal_approx_fast(rec, den)
                # sigmoid(eps/10) = 1 / (1 + exp(-eps/10))
                nege = tailp.tile([128, GT, DIM], f32, tag="nege", name="nege")
                nc.scalar.activation(nege, epp, Act.Exp, scale=-0.1)
                sd = tailp.tile([128, GT, DIM], f32, tag="sd", name="sd")
                nc.vector.tensor_scalar_add(sd, nege, 1.0)
                sig = tailp.tile([128, GT, DIM], f32, tag="sig", name="sig")
                nc.vector.reciprocal_approx_fast(sig, sd)
                ea = tailp.tile([128, GT, DIM], f32, tag="ea", name="ea")
                nc.scalar.activation(ea, alp, Act.Exp, scale=0.1)
                nums = tailp.tile([128, GT, DIM], f32, tag="nums", name="nums")
                nc.vector.tensor_sub(nums, num4, s2p)
                frac = tailp.tile([128, GT, DIM], f32, tag="frac", name="frac")
                nc.vector.tensor_mul(frac, nums, rec)
                q = tailp.tile([128, GT, DIM], f32, tag="q", name="q")
                nc.vector.scalar_tensor_tensor(
                    q, in0=frac, scalar=0.8, in1=sig, op0=Alu.mult, op1=Alu.mult)
                # y = ea*(x+q) + b2; sx per tile since x lives per-tile
                for t in range(GT):
                    it = g * GT + t
                    xf, y_out = xfs[it], youts[it]
                    xc = xf[:, s * DIM:(s + 1) * DIM]
                    sx = tailp.tile([128, DIM], f32, tag="sx", name="sx")
                    nc.vector.tensor_add(sx, q[:, t, :], xc)
                    yp = tailp.tile([128, DIM], f32, tag="yp", name="yp")
                    nc.vector.tensor_mul(yp, ea[:, t, :], sx)
                    nc.vector.tensor_add(y_out[:, s * DIM:(s + 1) * DIM],
                                         yp, b2p[:, t, :])
                if s == 1:
                    for t in range(GT):
                        it = g * GT + t
                        nc.sync.dma_start(y_d[it * 128:(it + 1) * 128, :],
                                          youts[it])

            # ---- software-pipelined emission over all chunks ----
            jobs = []
            for s in range(2):
                for it in range(NT):
                    for c in range(NCHUNK):
                        jobs.append({"s": s, "it": it, "c": c})
            n = len(jobs)
            for i in range(n + 4):
                if 0 <= i - 3 < n:
                    j = jobs[i - 3]
                    stage_e(j["s"], j["it"], j["c"], j)
                if i < n:
                    j = jobs[i]
                    if j["c"] == 0:
                        prelude(j["s"], j["it"])
                    stage_a(j["s"], j["it"], j["c"], j)
                if i - 1 >= 0 and i - 1 < n:
                    j = jobs[i - 1]
                    stage_b(j["s"], j["it"], j["c"], j)
                if i - 2 >= 0 and i - 2 < n:
                    j = jobs[i - 2]
                    stage_c(j["s"], j["it"], j["c"], j)
                if i - 4 >= 0:
                    j = jobs[i - 4]
                    stage_d(j["s"], j["it"], j["c"], j)
                    # tail after the last chunk of each 4-tile group
                    if j["c"] == NCHUNK - 1 and j["it"] % GT == GT - 1:
                        tail_group(j["s"], j["it"] // GT)
                    # release per-job tile references
                    for k in ("m1f", "m2f", "b1f", "u", "z1", "e", "tr0",
                              "th2"):
                        j.pop(k, None)

    nc.compile()
    return nc


def _prep_weights(W1, b1, W2, b2):
    w1a = np.concatenate([W1, b1[None, :]], axis=0).astype(np.float16)  # [33, 50]
    ones_col = np.zeros((DIM + 1, 1), dtype=np.float16)
    ones_col[DIM, 0] = 1.0
    w1a = np.concatenate([w1a, ones_col], axis=1)                       # [33, 51]
    w2a = np.concatenate([W2, b2[None, :]], axis=0)                     # [51, 9696] f32
    w2a = w2a.copy()
    w2a[50, DL:2 * DL] += 1.0   # bias1 region delivers b1+1 (see w = max(z+1, e))
    # append S2 columns: S2[:, d] = sum_l w2a[:, mat2 region (d, l)]
    m2cols = w2a[:, 2 * DL:3 * DL].reshape(51, DIM, LS)
    s2 = m2cols.sum(axis=2)                                             # [51, DIM]
    w2a = np.concatenate([w2a, s2], axis=1).astype(np.float16)          # [51, 9728]
    return np.ascontiguousarray(w1a), np.ascontiguousarray(w2a)


def kernel(**inputs):
    from concourse.bass_utils import run_bass_kernel_spmd

    if "nc" not in _cache:
        _cache["nc"] = _build_program()
    nc = _cache["nc"]

    x = np.ascontiguousarray(inputs["x"], dtype=np.float32)
    w1a1, w2a1 = _prep_weights(inputs["s1_W1"], inputs["s1_b1"],
                               inputs["s1_W2"], inputs["s1_b2"])
    w1a2, w2a2 = _prep_weights(inputs["s2_W1"], inputs["s2_b1"],
                               inputs["s2_W2"], inputs["s2_b2"])

    in_maps = []
    for i in range(NCORES):
        in_maps.append({
            "x": x[i * BC:(i + 1) * BC],
            "w1a1": w1a1, "w2a1": w2a1,
            "w1a2": w1a2, "w2a2": w2a2,
        })

    last_err = None
    for attempt in range(3):
        try:
            res = run_bass_kernel_spmd(nc, in_maps, core_ids=list(range(NCORES)),
                                       **_cache.get("run_kwargs", {}))
            out = np.concatenate([r["y"] for r in res.results], axis=0)
            _cache["last_results"] = res
            return out
        except Exception as ex:  # transient NRT/device errors: retry
            last_err = ex
    raise last_err


# revision 12
# speedup vs baseline: 1.1013x; 1.1013x over previous
"""Trainium2 Bass kernel for the CN coupling-block problem (nn_CN_69312182223156).

Math (per subnet s on half-features x_s with conditioner c):
    h   = relu(c @ W1 + b1)                       # [B, 50]
    p   = h @ W2 + b2                             # [B, 9696]
    m1, b1p, m2 = p[:, :3200], p[:, 3200:6400], p[:, 6400:9600]   (viewed [B,32,100])
    bias2, eps, alpha = p[:, 9600:9632], p[:, 9632:9664]/10, p[:, 9664:]/10
    z   = x*m1 + b1p
    num = sum_l elu(z)*m2 ;  den = sum_l relu(-m1*m2) + 1
    y   = exp(alpha) * (x + 0.8*sigmoid(eps)*num/den) + bias2

Subnet 1: x=x1, c=x2.  Subnet 2: x=x2, c=y1.  Output concat([y1, y2]).

Strategy: pure data-parallel over 8 cores (2048 rows each), weights replicated.
Layout: batch on SBUF partitions (tiles of 128 rows); the [B, 9696] parameter
tensor is produced on PE in 800-column chunks (8 dims x 100) and consumed
immediately.  Work is spread over three engines (the old version used two):
  ACT: PSUM->SBUF f16 casts (m1+m2 merged into one strided op, then b1) + exp
  DVE: per-dim x-broadcast tensor_scalar (4x), z1 add, min, max, w*m2, the
       final reduce, and the den reduction as 8 per-dim
       tensor_scalar(min 0, add 0) with fused accum_out (the hardware
       accumulator applies op1, so op1 must be the add; the -1 is applied in
       the tail).
  GPSIMD: u = m1*m2 and the two num pair-fold adds (only plain TT-class SBUF
       ops lower on Pool).
The per-chunk work is emitted as a 5-deep software pipeline (stages A-E over
chunk index) so that every instruction is data-ready when it reaches the head
of its engine's in-order queue: A(i)=matmuls+casts+u, B(i-1)=z-chain+exp,
C(i-2)=w,w*m2, E(i-3)=folds, D(i-4)=reduce+den.  Tails (bias2|eps|alpha|S2,
num/den combine) are batched over groups of 4 row-tiles.
Biases fold into augmented weights; S2 = sum_l mat2 rides as 32 extra matmul
columns so num needs no -1 term:  elu(z)+1 = max(z+1, exp(min(z,0))).
"""

import numpy as np

B = 16384
DIM = 32
LS = 100
NCORES = 8
BC = B // NCORES          # rows per core
NT = BC // 128            # 128-row tiles per core
GT = 4                    # tiles per tail group
DL = DIM * LS             # 3200
PW = 3 * DL + 3 * DIM     # 9696 params per row
CHUNK = 800               # params per elementwise chunk (8 dims x 100)
HALF = 400                # params per PSUM-bank matmul
NCHUNK = DL // CHUNK      # 4
DPC = CHUNK // LS         # 8 dims per chunk

_cache = {}


def _build_program():
    import concourse.bass as bass
    import concourse.tile as tile
    import concourse.mybir as mybir
    from concourse import bacc, masks

    f32 = mybir.dt.float32
    f16 = mybir.dt.float16
    Alu = mybir.AluOpType
    Act = mybir.ActivationFunctionType
    X = mybir.AxisListType.X

    nc = bacc.Bacc("TRN2", target_bir_lowering=False)

    x_d = nc.dram_tensor("x", [BC, 2 * DIM], f32, kind="ExternalInput")
    w1a = [nc.dram_tensor(f"w1a{s}", [DIM + 1, 51], f16, kind="ExternalInput")
           for s in (1, 2)]
    w2a = [nc.dram_tensor(f"w2a{s}", [51, PW + DIM], f16, kind="ExternalInput")
           for s in (1, 2)]
    y_d = nc.dram_tensor("y", [BC, 2 * DIM], f32, kind="ExternalOutput")

    with tile.TileContext(nc) as tc:
        with (
            tc.tile_pool(name="const", bufs=1) as const,
            tc.tile_pool(name="per", bufs=1) as per,
            tc.tile_pool(name="mid", bufs=4) as mid,
            tc.tile_pool(name="hts", bufs=8) as hts,
            tc.tile_pool(name="ew", bufs=7) as ew,
            tc.tile_pool(name="tailp", bufs=2) as tailp,
            tc.tile_pool(name="pmm1", bufs=1, space="PSUM") as pmm1,
            tc.tile_pool(name="pmm2", bufs=1, space="PSUM") as pmm2,
            tc.tile_pool(name="psm", bufs=2, space="PSUM") as psm,
        ):
            # ---- constants ----
            w1s = []
            w2s = []
            for s in range(2):
                t1 = const.tile([DIM + 1, 51], f16, tag=f"w1_{s}", name="t1")
                nc.sync.dma_start(t1, w1a[s][:])
                w1s.append(t1)
                t2 = const.tile([51, PW + DIM], f16, tag=f"w2_{s}", name="t2")
                nc.sync.dma_start(t2, w2a[s][:])
                w2s.append(t2)
            identf = const.tile([128, 128], f32, tag="identf", name="identf")
            masks.make_identity(nc, identf[:])
            negone = const.tile([128, 1], f32, tag="negone", name="negone")
            nc.vector.memset(negone, -1.0)

            xfs, youts, nd4s, hTd = {}, {}, {}, {}

            def prelude(s, it):
                """Per-tile setup: x load (phase 1), conditioner transpose,
                h matmul, hT relu-cast."""
                if s == 0:
                    r0 = it * 128
                    xf = per.tile([128, 2 * DIM + 1], f32, tag=f"xf{it}",
                                  name="xf")
                    nc.sync.dma_start(xf[:, 0:2 * DIM], x_d[r0:r0 + 128, :])
                    nc.vector.memset(xf[:, 2 * DIM:], 1.0)
                    xfs[it] = xf
                    y_out = per.tile([128, 2 * DIM], f32, tag=f"y_out{it}",
                                     name="y_out")
                    youts[it] = y_out
                    # conditioner for subnet 1: [x2 | 1]^T  -> [33, 128]
                    ct_ps = psm.tile([DIM + 1, 128], f32, tag="tp", name="ct_ps")
                    nc.tensor.transpose(ct_ps, xf[:, DIM:2 * DIM + 1], identf)
                    condT = mid.tile([DIM + 1, 128], f16, tag="condT",
                                     name="condT")
                    nc.scalar.copy(condT, ct_ps)
                else:
                    y_out = youts[it]
                    # conditioner for subnet 2: [y1 | 1]^T
                    c2_ps = psm.tile([DIM, 128], f32, tag="tp", name="c2_ps")
                    nc.tensor.transpose(c2_ps, y_out[:, 0:DIM], identf)
                    condT = mid.tile([DIM + 1, 128], f16, tag="condT",
                                     name="condT2")
                    nc.scalar.copy(condT[0:DIM, :], c2_ps)
                    nc.vector.memset(condT[DIM:DIM + 1, :], 1.0)
                # h^T = relu(W1^T c^T + b1): [51, 128]; col 50 of W1aug is
                # e_32 so row 50 comes out as relu(1) = 1 (the aug ones row).
                h_ps = psm.tile([51, 128], f32, tag="tp", name="h_ps")
                nc.tensor.matmul(h_ps, w1s[s], condT, start=True, stop=True)
                hT = hts.tile([51, 128], f16, tag="hT", name="hT")
                nc.scalar.activation(hT, h_ps, Act.Relu)
                hTd[(s, it)] = hT
                g, itg = it // GT, it % GT
                if itg == 0:
                    nd4s[(s, g)] = per.tile([128, GT, 2, DIM], f32,
                                            tag=f"nd{s}_{g}", name="nd4")

            def stage_a(s, it, c, st):
                """PE matmuls into PSUM, ACT casts (m1+m2 first so the Pool's
                u can start early, then b1), Pool u = m1*m2."""
                hT = hTd[(s, it)]
                co = c * CHUNK
                p1 = pmm1.tile([128, 4, 512], f32, tag="p1", name="p1")
                p2 = pmm2.tile([128, 2, 512], f32, tag="p2", name="p2")
                for hh in range(2):
                    o = co + hh * HALF
                    nc.tensor.matmul(p1[:, hh, 0:HALF], hT,
                                     w2s[s][:, o:o + HALF],
                                     start=True, stop=True)
                    nc.tensor.matmul(p1[:, 2 + hh, 0:HALF], hT,
                                     w2s[s][:, 2 * DL + o:2 * DL + o + HALF],
                                     start=True, stop=True)
                for hh in range(2):
                    o = co + hh * HALF
                    nc.tensor.matmul(p2[:, hh, 0:HALF], hT,
                                     w2s[s][:, DL + o:DL + o + HALF],
                                     start=True, stop=True)
                mm12 = ew.tile([128, 4, HALF], f16, tag="mm12", name="mm12")
                nc.scalar.copy(mm12, p1[:, :, 0:HALF])
                b1t = ew.tile([128, 2, HALF], f16, tag="b1t", name="b1t")
                nc.scalar.copy(b1t, p2[:, :, 0:HALF])
                st["m1f"] = mm12[:, 0:2, :].rearrange("p h q -> p (h q)")
                st["m2f"] = mm12[:, 2:4, :].rearrange("p h q -> p (h q)")
                st["b1f"] = b1t.rearrange("p h q -> p (h q)")
                u = ew.tile([128, CHUNK], f16, tag="u", name="u")
                nc.gpsimd.tensor_mul(u, st["m1f"], st["m2f"])
                st["u"] = u

            def stage_b(s, it, c, st):
                """z = x*m1 (+ b1+1), capped arg, exp."""
                xf = xfs[it]
                xc32 = xf[:, s * DIM:(s + 1) * DIM]
                zmul = ew.tile([128, CHUNK], f16, tag="zmul", name="zmul")
                zm3 = zmul.rearrange("p (d l) -> p d l", l=LS)
                m1s3 = st["m1f"].rearrange("p (d l) -> p d l", l=LS)
                for j in range(DPC):
                    nc.vector.tensor_scalar_mul(
                        zm3[:, j, :], m1s3[:, j, :],
                        xc32[:, c * DPC + j:c * DPC + j + 1])
                # b1f carries b1+1, so z1 = z+1 and, using e^x >= 1+x:
                #   elu(z)+1 = max(z+1, exp(min(z,0)))
                z1 = ew.tile([128, CHUNK], f16, tag="z1", name="z1")
                nc.vector.tensor_add(z1, zmul, st["b1f"])
                zn = ew.tile([128, CHUNK], f16, tag="zn", name="zn")
                nc.vector.tensor_scalar_min(zn, z1, 1.0)
                e = ew.tile([128, CHUNK], f16, tag="e", name="e")
                nc.scalar.activation(e, zn, Act.Exp, bias=negone)
                st["z1"], st["e"] = z1, e

            def stage_c(s, it, c, st):
                """w = elu(z)+1 = max(z1, e);  t = w*m2."""
                w = ew.tile([128, CHUNK], f16, tag="w", name="w")
                nc.vector.tensor_tensor(w, st["z1"], st["e"], Alu.max)
                tr0 = ew.tile([128, DPC, LS], f16, tag="tr0", name="tr0")
                nc.vector.tensor_mul(
                    tr0, w.rearrange("p (d l) -> p d l", l=LS),
                    st["m2f"].rearrange("p (d l) -> p d l", l=LS))
                st["tr0"] = tr0

            def stage_e1(s, it, c, st):
                """First pair-fold add for num, on the Pool engine."""
                tr4 = st["tr0"].rearrange("p d (f l) -> p d f l", f=2)
                th = ew.tile([128, DPC, LS // 2], f16, tag="th", name="th")
                nc.gpsimd.tensor_add(th, tr4[:, :, 0, :], tr4[:, :, 1, :])
                st["th"] = th

            def stage_e2(s, it, c, st):
                """Second pair-fold add for num, on the Pool engine (one
                pipeline stage later so its input is a full iteration old and
                the Pool queue never stalls mid-chain)."""
                th4 = st["th"].rearrange("p d (f l) -> p d f l", f=2)
                th2 = ew.tile([128, DPC, LS // 4], f16, tag="th2", name="th2")
                nc.gpsimd.tensor_add(th2, th4[:, :, 0, :], th4[:, :, 1, :])
                st["th2"] = th2

            def stage_d(s, it, c, st):
                """Final num reduce; den via per-dim min/add with fused
                accumulate (accum applies op1=add; sign fixed in the tail:
                sum_l relu(-u) = -sum_l min(u, 0))."""
                g, itg = it // GT, it % GT
                nd4 = nd4s[(s, g)]
                numo = nd4[:, itg, 0, c * DPC:(c + 1) * DPC]
                nc.vector.tensor_reduce(numo, st["th2"], X, Alu.add)
                scr = ew.tile([128, DPC, LS], f16, tag="scr", name="scr")
                u3 = st["u"].rearrange("p (d l) -> p d l", l=LS)
                for j in range(DPC):
                    dd = c * DPC + j
                    nc.vector.tensor_scalar(
                        scr[:, j, :], u3[:, j, :], 0.0, 0.0,
                        Alu.min, Alu.add,
                        accum_out=nd4[:, itg, 1, dd:dd + 1])

            def tail_group(s, g):
                """Batched tail for GT tiles: tail matmuls into one PSUM bank,
                elementwise at FD=128 on [128, GT, DIM] views."""
                nd4 = nd4s[(s, g)]
                tp4 = psm.tile([128, 4, 128], f32, tag="tp", name="tp4")
                for t in range(GT):
                    nc.tensor.matmul(tp4[:, t, :], hTd[(s, g * GT + t)],
                                     w2s[s][:, 3 * DL:3 * DL + 4 * DIM],
                                     start=True, stop=True)
                b2p = tp4[:, :, 0:DIM]
                epp = tp4[:, :, DIM:2 * DIM]
                alp = tp4[:, :, 2 * DIM:3 * DIM]
                s2p = tp4[:, :, 3 * DIM:4 * DIM]
                num4 = nd4[:, :, 0, :]
                den4 = nd4[:, :, 1, :]

                den = tailp.tile([128, GT, DIM], f32, tag="den", name="den")
                nc.vector.tensor_scalar(den, den4, -1.0, 1.0, Alu.mult, Alu.add)
                rec = tailp.tile([128, GT, DIM], f32, tag="rec", name="rec")
                nc.vector.reciprocal_approx_fast(rec, den)
                # sigmoid(eps/10) = 1 / (1 + exp(-eps/10))
                nege = tailp.tile([128, GT, DIM], f32, tag="nege", name="nege")
                nc.scalar.activation(nege, epp, Act.Exp, scale=-0.1)
                sd = tailp.tile([128, GT, DIM], f32, tag="sd", name="sd")
                nc.vector.tensor_scalar_add(sd, nege, 1.0)
                sig = tailp.tile([128, GT, DIM], f32, tag="sig", name="sig")
                nc.vector.reciprocal_approx_fast(sig, sd)
                ea = tailp.tile([128, GT, DIM], f32, tag="ea", name="ea")
                nc.scalar.activation(ea, alp, Act.Exp, scale=0.1)
                nums = tailp.tile([128, GT, DIM], f32, tag="nums", name="nums")
                nc.vector.tensor_sub(nums, num4, s2p)
                frac = tailp.tile([128, GT, DIM], f32, tag="frac", name="frac")
                nc.vector.tensor_mul(frac, nums, rec)
                q = tailp.tile([128, GT, DIM], f32, tag="q", name="q")
                nc.vector.scalar_tensor_tensor(
                    q, in0=frac, scalar=0.8, in1=sig, op0=Alu.mult, op1=Alu.mult)
                # y = ea*(x+q) + b2; sx per tile since x lives per-tile
                for t in range(GT):
                    it = g * GT + t
                    xf, y_out = xfs[it], youts[it]
                    xc = xf[:, s * DIM:(s + 1) * DIM]
                    sx = tailp.tile([128, DIM], f32, tag="sx", name="sx")
                    nc.vector.tensor_add(sx, q[:, t, :], xc)
                    yp = tailp.tile([128, DIM], f32, tag="yp", name="yp")
                    nc.vector.tensor_mul(yp, ea[:, t, :], sx)
                    nc.vector.tensor_add(y_out[:, s * DIM:(s + 1) * DIM],
                                         yp, b2p[:, t, :])
                if s == 1:
                    for t in range(GT):
                        it = g * GT + t
                        nc.sync.dma_start(y_d[it * 128:(it + 1) * 128, :],
                                          youts[it])

            # ---- software-pipelined emission over all chunks ----
            jobs = []
            for s in range(2):
                for it in range(NT):
                    for c in range(NCHUNK):
                        jobs.append({"s": s, "it": it, "c": c})
            n = len(jobs)
            for i in range(n + 5):
                if 0 <= i - 4 < n:
                    j = jobs[i - 4]
                    stage_e2(j["s"], j["it"], j["c"], j)
                if 0 <= i - 3 < n:
                    j = jobs[i - 3]
                    stage_e1(j["s"], j["it"], j["c"], j)
                if i < n:
                    j = jobs[i]
                    if j["c"] == 0:
                        prelude(j["s"], j["it"])
                    stage_a(j["s"], j["it"], j["c"], j)
                if 0 <= i - 1 < n:
                    j = jobs[i - 1]
                    stage_b(j["s"], j["it"], j["c"], j)
                if 0 <= i - 2 < n:
                    j = jobs[i - 2]
                    stage_c(j["s"], j["it"], j["c"], j)
                if 0 <= i - 5 < n:
                    j = jobs[i - 5]
                    stage_d(j["s"], j["it"], j["c"], j)
                    # tail after the last chunk of each 4-tile group
                    if j["c"] == NCHUNK - 1 and j["it"] % GT == GT - 1:
                        tail_group(j["s"], j["it"] // GT)
                    # release per-job tile references
                    for k in ("m1f", "m2f", "b1f", "u", "z1", "e", "tr0",
                              "th", "th2"):
                        j.pop(k, None)

    nc.compile()
    return nc


def _prep_weights(W1, b1, W2, b2):
    w1a = np.concatenate([W1, b1[None, :]], axis=0).astype(np.float16)  # [33, 50]
    ones_col = np.zeros((DIM + 1, 1), dtype=np.float16)
    ones_col[DIM, 0] = 1.0
    w1a = np.concatenate([w1a, ones_col], axis=1)                       # [33, 51]
    w2a = np.concatenate([W2, b2[None, :]], axis=0)                     # [51, 9696] f32
    w2a = w2a.copy()
    w2a[50, DL:2 * DL] += 1.0   # bias1 region delivers b1+1 (see w = max(z+1, e))
    # append S2 columns: S2[:, d] = sum_l w2a[:, mat2 region (d, l)]
    m2cols = w2a[:, 2 * DL:3 * DL].reshape(51, DIM, LS)
    s2 = m2cols.sum(axis=2)                                             # [51, DIM]
    w2a = np.concatenate([w2a, s2], axis=1).astype(np.float16)          # [51, 9728]
    return np.ascontiguousarray(w1a), np.ascontiguousarray(w2a)


def kernel(**inputs):
    from concourse.bass_utils import run_bass_kernel_spmd

    if "nc" not in _cache:
        _cache["nc"] = _build_program()
    nc = _cache["nc"]

    x = np.ascontiguousarray(inputs["x"], dtype=np.float32)
    w1a1, w2a1 = _prep_weights(inputs["s1_W1"], inputs["s1_b1"],
                               inputs["s1_W2"], inputs["s1_b2"])
    w1a2, w2a2 = _prep_weights(inputs["s2_W1"], inputs["s2_b1"],
                               inputs["s2_W2"], inputs["s2_b2"])

    in_maps = []
    for i in range(NCORES):
        in_maps.append({
            "x": x[i * BC:(i + 1) * BC],
            "w1a1": w1a1, "w2a1": w2a1,
            "w1a2": w1a2, "w2a2": w2a2,
        })

    last_err = None
    for attempt in range(3):
        try:
            res = run_bass_kernel_spmd(nc, in_maps, core_ids=list(range(NCORES)),
                                       **_cache.get("run_kwargs", {}))
            out = np.concatenate([r["y"] for r in res.results], axis=0)
            _cache["last_results"] = res
            return out
        except Exception as ex:  # transient NRT/device errors: retry
            last_err = ex
    raise last_err


# revision 15
# speedup vs baseline: 1.1703x; 1.0627x over previous
"""Trainium2 Bass kernel for the CN coupling-block problem (nn_CN_69312182223156).

Math (per subnet s on half-features x_s with conditioner c):
    h   = relu(c @ W1 + b1)                       # [B, 50]
    p   = h @ W2 + b2                             # [B, 9696]
    m1, b1p, m2 = p[:, :3200], p[:, 3200:6400], p[:, 6400:9600]   (viewed [B,32,100])
    bias2, eps, alpha = p[:, 9600:9632], p[:, 9632:9664]/10, p[:, 9664:]/10
    z   = x*m1 + b1p
    num = sum_l elu(z)*m2 ;  den = sum_l relu(-m1*m2) + 1
    y   = exp(alpha) * (x + 0.8*sigmoid(eps)*num/den) + bias2

Subnet 1: x=x1, c=x2.  Subnet 2: x=x2, c=y1.  Output concat([y1, y2]).

Strategy: pure data-parallel over 8 cores (2048 rows each), weights replicated.
Layout: batch on SBUF partitions (tiles of 128 rows); the [B, 9696] parameter
tensor is produced on PE in 800-column chunks (8 dims x 100) and consumed
immediately.  Work is spread over three engines (the old version used two):
  ACT: PSUM->SBUF f16 casts (m1+m2 merged into one strided op, then b1) + exp
  DVE: per-dim x-broadcast tensor_scalar (4x), z1 add, min, max, w*m2, the
       final reduce, and the den reduction as 8 per-dim
       tensor_scalar(min 0, add 0) with fused accum_out (the hardware
       accumulator applies op1, so op1 must be the add; the -1 is applied in
       the tail).
  GPSIMD: u = m1*m2 and the two num pair-fold adds (only plain TT-class SBUF
       ops lower on Pool).
The per-chunk work is emitted as a 5-deep software pipeline (stages A-E over
chunk index) so that every instruction is data-ready when it reaches the head
of its engine's in-order queue: A(i)=matmuls+casts+u, B(i-1)=z-chain+exp,
C(i-2)=w,w*m2, E(i-3)=folds, D(i-4)=reduce+den.  Tails (bias2|eps|alpha|S2,
num/den combine) are batched over groups of 4 row-tiles.
Biases fold into augmented weights; S2 = sum_l mat2 rides as 32 extra matmul
columns so num needs no -1 term:  elu(z)+1 = max(z+1, exp(min(z,0))).
"""

import numpy as np

B = 16384
DIM = 32
LS = 100
NCORES = 8
BC = B // NCORES          # rows per core
NT = BC // 128            # 128-row tiles per core
GT = 4                    # tiles per tail group
DL = DIM * LS             # 3200
PW = 3 * DL + 3 * DIM     # 9696 params per row
CHUNK = 800               # params per elementwise chunk (8 dims x 100)
HALF = 400                # params per PSUM-bank matmul
NCHUNK = DL // CHUNK      # 4
DPC = CHUNK // LS         # 8 dims per chunk

_cache = {}


def _build_program():
    import concourse.bass as bass
    import concourse.tile as tile
    import concourse.mybir as mybir
    from concourse import bacc, masks

    f32 = mybir.dt.float32
    f16 = mybir.dt.float16
    Alu = mybir.AluOpType
    Act = mybir.ActivationFunctionType
    X = mybir.AxisListType.X

    nc = bacc.Bacc("TRN2", target_bir_lowering=False)

    x_d = nc.dram_tensor("x", [BC, 2 * DIM], f32, kind="ExternalInput")
    w1a = [nc.dram_tensor(f"w1a{s}", [DIM + 1, 51], f16, kind="ExternalInput")
           for s in (1, 2)]
    w2a = [nc.dram_tensor(f"w2a{s}", [51, PW + DIM], f16, kind="ExternalInput")
           for s in (1, 2)]
    y_d = nc.dram_tensor("y", [BC, 2 * DIM], f32, kind="ExternalOutput")

    with tile.TileContext(nc) as tc:
        with (
            tc.tile_pool(name="const", bufs=1) as const,
            tc.tile_pool(name="per", bufs=1) as per,
            tc.tile_pool(name="mid", bufs=4) as mid,
            tc.tile_pool(name="hts", bufs=8) as hts,
            tc.tile_pool(name="ew", bufs=7) as ew,
            tc.tile_pool(name="tailp", bufs=2) as tailp,
            tc.tile_pool(name="pmm1", bufs=1, space="PSUM") as pmm1,
            tc.tile_pool(name="pmm2", bufs=1, space="PSUM") as pmm2,
            tc.tile_pool(name="psm", bufs=2, space="PSUM") as psm,
        ):
            # ---- constants ----
            w1s = []
            w2s = []
            for s in range(2):
                t1 = const.tile([DIM + 1, 51], f16, tag=f"w1_{s}", name="t1")
                nc.sync.dma_start(t1, w1a[s][:])
                w1s.append(t1)
                t2 = const.tile([51, PW + DIM], f16, tag=f"w2_{s}", name="t2")
                nc.sync.dma_start(t2, w2a[s][:])
                w2s.append(t2)
            identf = const.tile([128, 128], f32, tag="identf", name="identf")
            masks.make_identity(nc, identf[:])
            negone = const.tile([128, 1], f32, tag="negone", name="negone")
            nc.vector.memset(negone, -1.0)

            xfs, youts, nd4s, hTd = {}, {}, {}, {}

            def prelude(s, it):
                """Per-tile setup: x load (phase 1), conditioner transpose,
                h matmul, hT relu-cast."""
                if s == 0:
                    r0 = it * 128
                    xf = per.tile([128, 2 * DIM + 1], f32, tag=f"xf{it}",
                                  name="xf")
                    nc.sync.dma_start(xf[:, 0:2 * DIM], x_d[r0:r0 + 128, :])
                    nc.vector.memset(xf[:, 2 * DIM:], 1.0)
                    xfs[it] = xf
                    y_out = per.tile([128, 2 * DIM], f32, tag=f"y_out{it}",
                                     name="y_out")
                    youts[it] = y_out
                    # conditioner for subnet 1: [x2 | 1]^T  -> [33, 128]
                    ct_ps = psm.tile([DIM + 1, 128], f32, tag="tp", name="ct_ps")
                    nc.tensor.transpose(ct_ps, xf[:, DIM:2 * DIM + 1], identf)
                    condT = mid.tile([DIM + 1, 128], f16, tag="condT",
                                     name="condT")
                    nc.scalar.copy(condT, ct_ps)
                else:
                    y_out = youts[it]
                    # conditioner for subnet 2: [y1 | 1]^T
                    c2_ps = psm.tile([DIM, 128], f32, tag="tp", name="c2_ps")
                    nc.tensor.transpose(c2_ps, y_out[:, 0:DIM], identf)
                    condT = mid.tile([DIM + 1, 128], f16, tag="condT",
                                     name="condT2")
                    nc.scalar.copy(condT[0:DIM, :], c2_ps)
                    nc.vector.memset(condT[DIM:DIM + 1, :], 1.0)
                # h^T = relu(W1^T c^T + b1): [51, 128]; col 50 of W1aug is
                # e_32 so row 50 comes out as relu(1) = 1 (the aug ones row).
                h_ps = psm.tile([51, 128], f32, tag="tp", name="h_ps")
                nc.tensor.matmul(h_ps, w1s[s], condT, start=True, stop=True)
                hT = hts.tile([51, 128], f16, tag="hT", name="hT")
                nc.scalar.activation(hT, h_ps, Act.Relu)
                hTd[(s, it)] = hT
                g, itg = it // GT, it % GT
                if itg == 0:
                    nd4s[(s, g)] = per.tile([128, GT, 2, DIM], f32,
                                            tag=f"nd{s}_{g}", name="nd4")

            def stage_a(s, it, c, st):
                """PE matmuls into PSUM, ACT casts (m1+m2 first so the Pool's
                u can start early, then b1), Pool u = m1*m2."""
                hT = hTd[(s, it)]
                co = c * CHUNK
                p1 = pmm1.tile([128, 4, 512], f32, tag="p1", name="p1")
                p2 = pmm2.tile([128, 2, 512], f32, tag="p2", name="p2")
                for hh in range(2):
                    o = co + hh * HALF
                    nc.tensor.matmul(p1[:, hh, 0:HALF], hT,
                                     w2s[s][:, o:o + HALF],
                                     start=True, stop=True)
                    nc.tensor.matmul(p1[:, 2 + hh, 0:HALF], hT,
                                     w2s[s][:, 2 * DL + o:2 * DL + o + HALF],
                                     start=True, stop=True)
                for hh in range(2):
                    o = co + hh * HALF
                    nc.tensor.matmul(p2[:, hh, 0:HALF], hT,
                                     w2s[s][:, DL + o:DL + o + HALF],
                                     start=True, stop=True)
                mm12 = ew.tile([128, 4, HALF], f16, tag="mm12", name="mm12")
                nc.scalar.copy(mm12, p1[:, :, 0:HALF])
                b1t = ew.tile([128, 2, HALF], f16, tag="b1t", name="b1t")
                nc.scalar.copy(b1t, p2[:, :, 0:HALF])
                st["m1f"] = mm12[:, 0:2, :].rearrange("p h q -> p (h q)")
                st["m2f"] = mm12[:, 2:4, :].rearrange("p h q -> p (h q)")
                st["b1f"] = b1t.rearrange("p h q -> p (h q)")
                u = ew.tile([128, CHUNK], f16, tag="u", name="u")
                nc.gpsimd.tensor_mul(u, st["m1f"], st["m2f"])
                st["u"] = u

            def stage_b(s, it, c, st):
                """z = x*m1 (+ b1+1), capped arg, exp."""
                xf = xfs[it]
                xc32 = xf[:, s * DIM:(s + 1) * DIM]
                zmul = ew.tile([128, CHUNK], f16, tag="zmul", name="zmul")
                zm3 = zmul.rearrange("p (d l) -> p d l", l=LS)
                m1s3 = st["m1f"].rearrange("p (d l) -> p d l", l=LS)
                for j in range(DPC):
                    nc.vector.tensor_scalar_mul(
                        zm3[:, j, :], m1s3[:, j, :],
                        xc32[:, c * DPC + j:c * DPC + j + 1])
                # b1f carries b1+1, so z1 = z+1 and, using e^x >= 1+x:
                #   elu(z)+1 = max(z+1, exp(min(z,0)))
                z1 = ew.tile([128, CHUNK], f16, tag="z1", name="z1")
                nc.vector.tensor_add(z1, zmul, st["b1f"])
                zn = ew.tile([128, CHUNK], f16, tag="zn", name="zn")
                nc.vector.tensor_scalar_min(zn, z1, 1.0)
                e = ew.tile([128, CHUNK], f16, tag="e", name="e")
                nc.scalar.activation(e, zn, Act.Exp, bias=negone)
                st["z1"], st["e"] = z1, e

            def stage_c(s, it, c, st):
                """w = elu(z)+1 = max(z1, e)."""
                w = ew.tile([128, CHUNK], f16, tag="w", name="w")
                nc.vector.tensor_tensor(w, st["z1"], st["e"], Alu.max)
                st["w"] = w

            def stage_c2(s, it, c, st):
                """t = w*m2 on the Pool engine (inputs one iteration old so
                the in-order Pool queue never head-blocks)."""
                tr0 = ew.tile([128, DPC, LS], f16, tag="tr0", name="tr0")
                nc.gpsimd.tensor_mul(
                    tr0, st["w"].rearrange("p (d l) -> p d l", l=LS),
                    st["m2f"].rearrange("p (d l) -> p d l", l=LS))
                st["tr0"] = tr0

            def stage_e(s, it, c, st):
                """Two pair-fold adds for num, on DVE (2x rate there)."""
                tr4 = st["tr0"].rearrange("p d (f l) -> p d f l", f=2)
                th = ew.tile([128, DPC, LS // 2], f16, tag="th", name="th")
                nc.vector.tensor_add(th, tr4[:, :, 0, :], tr4[:, :, 1, :])
                th4 = th.rearrange("p d (f l) -> p d f l", f=2)
                th2 = ew.tile([128, DPC, LS // 4], f16, tag="th2", name="th2")
                nc.vector.tensor_add(th2, th4[:, :, 0, :], th4[:, :, 1, :])
                st["th2"] = th2

            def stage_d(s, it, c, st):
                """Final num reduce; den via per-dim min/add with fused
                accumulate (accum applies op1=add; sign fixed in the tail:
                sum_l relu(-u) = -sum_l min(u, 0))."""
                g, itg = it // GT, it % GT
                nd4 = nd4s[(s, g)]
                numo = nd4[:, itg, 0, c * DPC:(c + 1) * DPC]
                nc.vector.tensor_reduce(numo, st["th2"], X, Alu.add)
                scr = ew.tile([128, DPC, LS], f16, tag="scr", name="scr")
                u3 = st["u"].rearrange("p (d l) -> p d l", l=LS)
                for j in range(DPC):
                    dd = c * DPC + j
                    nc.vector.tensor_scalar(
                        scr[:, j, :], u3[:, j, :], 0.0, 0.0,
                        Alu.min, Alu.add,
                        accum_out=nd4[:, itg, 1, dd:dd + 1])

            def tail_group(s, g):
                """Batched tail for GT tiles: tail matmuls into one PSUM bank,
                elementwise at FD=128 on [128, GT, DIM] views."""
                nd4 = nd4s[(s, g)]
                tp4 = psm.tile([128, 4, 128], f32, tag="tp", name="tp4")
                for t in range(GT):
                    nc.tensor.matmul(tp4[:, t, :], hTd[(s, g * GT + t)],
                                     w2s[s][:, 3 * DL:3 * DL + 4 * DIM],
                                     start=True, stop=True)
                b2p = tp4[:, :, 0:DIM]
                epp = tp4[:, :, DIM:2 * DIM]
                alp = tp4[:, :, 2 * DIM:3 * DIM]
                s2p = tp4[:, :, 3 * DIM:4 * DIM]
                num4 = nd4[:, :, 0, :]
                den4 = nd4[:, :, 1, :]

                den = tailp.tile([128, GT, DIM], f32, tag="den", name="den")
                nc.vector.tensor_scalar(den, den4, -1.0, 1.0, Alu.mult, Alu.add)
                rec = tailp.tile([128, GT, DIM], f32, tag="rec", name="rec")
                nc.vector.reciprocal_approx_fast(rec, den)
                # sigmoid(eps/10) = 1 / (1 + exp(-eps/10))
                nege = tailp.tile([128, GT, DIM], f32, tag="nege", name="nege")
                nc.scalar.activation(nege, epp, Act.Exp, scale=-0.1)
                sd = tailp.tile([128, GT, DIM], f32, tag="sd", name="sd")
                nc.vector.tensor_scalar_add(sd, nege, 1.0)
                sig = tailp.tile([128, GT, DIM], f32, tag="sig", name="sig")
                nc.vector.reciprocal_approx_fast(sig, sd)
                ea = tailp.tile([128, GT, DIM], f32, tag="ea", name="ea")
                nc.scalar.activation(ea, alp, Act.Exp, scale=0.1)
                nums = tailp.tile([128, GT, DIM], f32, tag="nums", name="nums")
                nc.vector.tensor_sub(nums, num4, s2p)
                frac = tailp.tile([128, GT, DIM], f32, tag="frac", name="frac")
                nc.vector.tensor_mul(frac, nums, rec)
                q = tailp.tile([128, GT, DIM], f32, tag="q", name="q")
                nc.vector.scalar_tensor_tensor(
                    q, in0=frac, scalar=0.8, in1=sig, op0=Alu.mult, op1=Alu.mult)
                # y = ea*(x+q) + b2; sx per tile since x lives per-tile
                for t in range(GT):
                    it = g * GT + t
                    xf, y_out = xfs[it], youts[it]
                    xc = xf[:, s * DIM:(s + 1) * DIM]
                    sx = tailp.tile([128, DIM], f32, tag="sx", name="sx")
                    nc.vector.tensor_add(sx, q[:, t, :], xc)
                    yp = tailp.tile([128, DIM], f32, tag="yp", name="yp")
                    nc.vector.tensor_mul(yp, ea[:, t, :], sx)
                    nc.vector.tensor_add(y_out[:, s * DIM:(s + 1) * DIM],
                                         yp, b2p[:, t, :])
                if s == 1:
                    for t in range(GT):
                        it = g * GT + t
                        nc.sync.dma_start(y_d[it * 128:(it + 1) * 128, :],
                                          youts[it])

            # ---- software-pipelined emission over all chunks ----
            jobs = []
            for s in range(2):
                for it in range(NT):
                    for c in range(NCHUNK):
                        jobs.append({"s": s, "it": it, "c": c})
            n = len(jobs)
            for i in range(n + 5):
                if 0 <= i - 3 < n:
                    j = jobs[i - 3]
                    stage_c2(j["s"], j["it"], j["c"], j)
                if i < n:
                    j = jobs[i]
                    if j["c"] == 0:
                        prelude(j["s"], j["it"])
                    stage_a(j["s"], j["it"], j["c"], j)
                if 0 <= i - 1 < n:
                    j = jobs[i - 1]
                    stage_b(j["s"], j["it"], j["c"], j)
                if 0 <= i - 2 < n:
                    j = jobs[i - 2]
                    stage_c(j["s"], j["it"], j["c"], j)
                if 0 <= i - 4 < n:
                    j = jobs[i - 4]
                    stage_e(j["s"], j["it"], j["c"], j)
                if 0 <= i - 5 < n:
                    j = jobs[i - 5]
                    stage_d(j["s"], j["it"], j["c"], j)
                    # tail after the last chunk of each 4-tile group
                    if j["c"] == NCHUNK - 1 and j["it"] % GT == GT - 1:
                        tail_group(j["s"], j["it"] // GT)
                    # release per-job tile references
                    for k in ("m1f", "m2f", "b1f", "u", "z1", "e", "w",
                              "tr0", "th2"):
                        j.pop(k, None)

    nc.compile()
    return nc


def _prep_weights(W1, b1, W2, b2):
    w1a = np.concatenate([W1, b1[None, :]], axis=0).astype(np.float16)  # [33, 50]
    ones_col = np.zeros((DIM + 1, 1), dtype=np.float16)
    ones_col[DIM, 0] = 1.0
    w1a = np.concatenate([w1a, ones_col], axis=1)                       # [33, 51]
    w2a = np.concatenate([W2, b2[None, :]], axis=0)                     # [51, 9696] f32
    w2a = w2a.copy()
    w2a[50, DL:2 * DL] += 1.0   # bias1 region delivers b1+1 (see w = max(z+1, e))
    # append S2 columns: S2[:, d] = sum_l w2a[:, mat2 region (d, l)]
    m2cols = w2a[:, 2 * DL:3 * DL].reshape(51, DIM, LS)
    s2 = m2cols.sum(axis=2)                                             # [51, DIM]
    w2a = np.concatenate([w2a, s2], axis=1).astype(np.float16)          # [51, 9728]
    return np.ascontiguousarray(w1a), np.ascontiguousarray(w2a)


def kernel(**inputs):
    from concourse.bass_utils import run_bass_kernel_spmd

    if "nc" not in _cache:
        _cache["nc"] = _build_program()
    nc = _cache["nc"]

    x = np.ascontiguousarray(inputs["x"], dtype=np.float32)
    w1a1, w2a1 = _prep_weights(inputs["s1_W1"], inputs["s1_b1"],
                               inputs["s1_W2"], inputs["s1_b2"])
    w1a2, w2a2 = _prep_weights(inputs["s2_W1"], inputs["s2_b1"],
                               inputs["s2_W2"], inputs["s2_b2"])

    in_maps = []
    for i in range(NCORES):
        in_maps.append({
            "x": x[i * BC:(i + 1) * BC],
            "w1a1": w1a1, "w2a1": w2a1,
            "w1a2": w1a2, "w2a2": w2a2,
        })

    last_err = None
    for attempt in range(3):
        try:
            res = run_bass_kernel_spmd(nc, in_maps, core_ids=list(range(NCORES)),
                                       **_cache.get("run_kwargs", {}))
            out = np.concatenate([r["y"] for r in res.results], axis=0)
            _cache["last_results"] = res
            return out
        except Exception as ex:  # transient NRT/device errors: retry
            last_err = ex
    raise last_err
